# revision 3
# baseline (speedup 1.0000x reference)
"""Trainium2 Bass kernel for nn_CausalConvolution (depthwise causal conv1d
+ bias + silu + attention mask, prefill path with conv-state cache output).

Reference computation (fp32):
    x  = hidden_states * mask[:, :, None]          # [B, S, C]
    xT = x.transpose(0, 2, 1)                      # [B, C, S]
    input_state = xT[..., -K:]                     # [B, C, K]
    y  = causal_depthwise_conv(xT, W) + b          # [B, C, S]
    y  = silu(y).transpose(0, 2, 1) * mask[..., None]

Sharding: 8 cores = (batch b in 0..3) x (channel half h in 0..1).  Depthwise
conv is independent per channel so this needs no communication.  Each core
processes x[b, h*1024:(h+1)*1024, :] in the [C, S] (channel-major) layout so
the per-channel filter taps become per-partition scalars:

    acc = x[:, t-3]*w0 + bias          (tensor_scalar,        VectorE)
    acc = x[:, t-2]*w1 + acc           (scalar_tensor_tensor, VectorE)
    acc = x[:, t-1]*w2 + acc           (scalar_tensor_tensor, VectorE)
    acc = x[:, t  ]*w3 + acc           (scalar_tensor_tensor, VectorE)
    y   = silu(acc)                    (activation,           ScalarE)

All arithmetic is fp32; the only deviations from the reference are fp32
summation order and ScalarE's spline silu (~1e-7 rel).

The host marshals [B,S,C] -> per-core [C_core, S] contiguous slices (plain
numpy transpose), feeds the 8 NeuronCores SPMD, and reassembles.  The conv
state cache is an exact slice of the transposed input, produced host-side.
"""

import os
import numpy as np

# Problem constants (hardcoded per harness contract; kernel.py must be
# self-contained).
B, S, C, K = 4, 4096, 2048, 4
N_CORES = 8
CH_HALVES = 2
CPC = C // CH_HALVES          # 1024 channels per core
P = 128                        # SBUF partitions
NBLK = CPC // P                # 8 channel blocks per core
PAD = K - 1                    # causal left padding

_cached = {}


def _build_nc():
    import concourse.bacc as bacc
    import concourse.mybir as mybir
    from concourse.tile import TileContext

    f32 = mybir.dt.float32
    nc = bacc.Bacc(None, target_bir_lowering=False)
    x_in = nc.dram_tensor("x", [CPC, S], f32, kind="ExternalInput")
    w_in = nc.dram_tensor("w", [CPC, K], f32, kind="ExternalInput")
    b_in = nc.dram_tensor("bias", [CPC, 1], f32, kind="ExternalInput")
    y_out = nc.dram_tensor("y", [CPC, S], f32, kind="ExternalOutput")

    with TileContext(nc) as tc:
        with (
            tc.tile_pool(name="xp", bufs=2) as xp,
            tc.tile_pool(name="wp", bufs=2) as wp,
            tc.tile_pool(name="ap", bufs=4) as ap,
            tc.tile_pool(name="yp", bufs=2) as yp,
        ):
            for blk in range(NBLK):
                c0 = blk * P
                cs = slice(c0, c0 + P)
                xt = xp.tile([P, PAD + S], f32, tag="xt")
                # causal left pad: x[:, t<0] = 0
                nc.vector.memset(xt[:, 0:PAD], 0.0)
                nc.sync.dma_start(out=xt[:, PAD:], in_=x_in[cs, :])
                wt = wp.tile([P, K], f32, tag="wt")
                bt = wp.tile([P, 1], f32, tag="bt")
                nc.sync.dma_start(out=wt[:], in_=w_in[cs, :])
                nc.sync.dma_start(out=bt[:], in_=b_in[cs, :])

                acc0 = ap.tile([P, S], f32, tag="acc")
                nc.vector.tensor_scalar(
                    acc0[:], xt[:, 0:S], wt[:, 0:1], bt[:, 0:1],
                    mybir.AluOpType.mult, mybir.AluOpType.add,
                )
                acc = acc0
                for k in range(1, K):
                    nxt = ap.tile([P, S], f32, tag="acc")
                    nc.vector.scalar_tensor_tensor(
                        nxt[:], xt[:, k:k + S], wt[:, k:k + 1], acc[:],
                        mybir.AluOpType.mult, mybir.AluOpType.add,
                    )
                    acc = nxt

                yt = yp.tile([P, S], f32, tag="yt")
                nc.scalar.activation(
                    yt[:], acc[:], mybir.ActivationFunctionType.Silu
                )
                nc.sync.dma_start(out=y_out[cs, :], in_=yt[:])
    nc.finalize()
    return nc


def _get_nc():
    if "nc" not in _cached:
        _cached["nc"] = _build_nc()
    return _cached["nc"]


def _install_ntff_shim():
    """Provide antenv.axon_hooks (absent in this container) so
    run_bass_kernel_spmd(trace=True) can NTFF-profile via the axon .so."""
    import sys
    import types
    if "antenv.axon_hooks" in sys.modules:
        return
    boot_dir = "/root/.axon_site/trn_agent_boot"
    so_path = "/opt/axon/libaxon_pjrt.so"
    if boot_dir not in sys.path:
        sys.path.insert(0, boot_dir)
    try:
        import trn_boot
        hook = trn_boot._ntff_profile_via_ctypes(so_path)
    except Exception:
        hook = None
    mod = types.ModuleType("antenv.axon_hooks")
    state = {"hook": hook}
    mod.set_axon_ntff_profile_hook = lambda h: state.update(hook=h)
    mod.get_axon_ntff_profile_hook = lambda: state["hook"]
    sys.modules["antenv.axon_hooks"] = mod


def kernel(hidden_states, attention_mask, W, b):
    from concourse.bass_utils import run_bass_kernel_spmd

    hidden_states = np.asarray(hidden_states, dtype=np.float32)
    attention_mask = np.asarray(attention_mask, dtype=np.float32)
    W = np.asarray(W, dtype=np.float32)
    b = np.asarray(b, dtype=np.float32)

    mask_is_ones = bool(np.all(attention_mask == 1.0))
    x = hidden_states if mask_is_ones else hidden_states * attention_mask[:, :, None]

    # [B, S, C] -> [B, C, S] contiguous for channel-major per-core slices
    xT = np.ascontiguousarray(x.transpose(0, 2, 1))
    input_state = np.ascontiguousarray(xT[:, :, S - K:])  # [B, C, K] exact

    w2d = np.ascontiguousarray(W.reshape(C, K))
    b2d = np.ascontiguousarray(b.reshape(C, 1))

    in_maps = []
    for core in range(N_CORES):
        bi, h = divmod(core, CH_HALVES)
        ch = slice(h * CPC, (h + 1) * CPC)
        in_maps.append({
            "x": np.ascontiguousarray(xT[bi, ch, :]),
            "w": w2d[ch],
            "bias": b2d[ch],
        })

    nc = _get_nc()
    trace = bool(int(os.environ.get("KERNEL_TRACE", "0")))
    if trace:
        _install_ntff_shim()
    res = run_bass_kernel_spmd(
        nc, in_maps, core_ids=list(range(N_CORES)), trace=trace,
        **({"trace_cores": list(range(N_CORES))} if trace else {}),
    )
    _cached["last_result"] = res

    yT = np.empty((B, C, S), dtype=np.float32)
    for core in range(N_CORES):
        bi, h = divmod(core, CH_HALVES)
        yT[bi, h * CPC:(h + 1) * CPC, :] = res.results[core]["y"]
    y = np.ascontiguousarray(yT.transpose(0, 2, 1))
    if not mask_is_ones:
        y *= attention_mask[:, :, None]
    return y, input_state


# revision 6
# speedup vs baseline: 1.2306x; 1.2306x over previous
"""Trainium2 Bass kernel for nn_CausalConvolution (depthwise causal conv1d
+ bias + silu + attention mask, prefill path with conv-state cache output).

Reference computation (fp32):
    x  = hidden_states * mask[:, :, None]          # [B, S, C]
    xT = x.transpose(0, 2, 1)                      # [B, C, S]
    input_state = xT[..., -K:]                     # [B, C, K]
    y  = causal_depthwise_conv(xT, W) + b          # [B, C, S]
    y  = silu(y).transpose(0, 2, 1) * mask[..., None]

Sharding: 8 cores = (batch b in 0..3) x (channel half h in 0..1).  Depthwise
conv is independent per channel so this needs no communication.  Each core
processes x[b, h*1024:(h+1)*1024, :] in the [C, S] (channel-major) layout so
the per-channel filter taps become per-partition scalars:

    acc = x[:, t-3]*w0 + bias          (tensor_scalar,        VectorE)
    acc = x[:, t-2]*w1 + acc           (scalar_tensor_tensor, VectorE)
    acc = x[:, t-1]*w2 + acc           (scalar_tensor_tensor, VectorE)
    acc = x[:, t  ]*w3 + acc           (scalar_tensor_tensor, VectorE)
    y   = silu(acc)                    (activation,           ScalarE)

All arithmetic is fp32; the only deviations from the reference are fp32
summation order and ScalarE's spline silu (~1e-7 rel).

The host marshals [B,S,C] -> per-core [C_core, S] contiguous slices (plain
numpy transpose), feeds the 8 NeuronCores SPMD, and reassembles.  The conv
state cache is an exact slice of the transposed input, produced host-side.
"""

import os
import numpy as np

# Problem constants (hardcoded per harness contract; kernel.py must be
# self-contained).
B, S, C, K = 4, 4096, 2048, 4
N_CORES = 8
CH_HALVES = 2
CPC = C // CH_HALVES          # 1024 channels per core
P = 128                        # SBUF partitions
NBLK = CPC // P                # 8 channel blocks per core
PAD = K - 1                    # causal left padding

_cached = {}


def _build_nc():
    import concourse.bacc as bacc
    import concourse.mybir as mybir
    from concourse.tile import TileContext

    f32 = mybir.dt.float32
    mult = mybir.AluOpType.mult
    add = mybir.AluOpType.add
    nc = bacc.Bacc(None, target_bir_lowering=False)
    # x comes pre-padded from the host: [CPC, PAD + S], first PAD cols zero
    x_in = nc.dram_tensor("x", [CPC, PAD + S], f32, kind="ExternalInput")
    w_in = nc.dram_tensor("w", [CPC, K], f32, kind="ExternalInput")
    b_in = nc.dram_tensor("bias", [CPC, 1], f32, kind="ExternalInput")
    y_out = nc.dram_tensor("y", [CPC, S], f32, kind="ExternalOutput")

    T = 2048                   # seq-chunk per tile
    NCHUNK = S // T
    with TileContext(nc) as tc:
        with (
            tc.tile_pool(name="xp", bufs=3) as xp,
            tc.tile_pool(name="wp", bufs=2) as wp,
            tc.tile_pool(name="ap", bufs=2) as ap,
            tc.tile_pool(name="yp", bufs=3) as yp,
        ):
            for blk in range(NBLK):
                c0 = blk * P
                cs = slice(c0, c0 + P)
                wt = wp.tile([P, K], f32, tag="wt")
                bt = wp.tile([P, 1], f32, tag="bt")
                nc.sync.dma_start(out=wt[:], in_=w_in[cs, :])
                nc.sync.dma_start(out=bt[:], in_=b_in[cs, :])
                for ci in range(NCHUNK):
                    t0 = ci * T
                    xt = xp.tile([P, PAD + T], f32, tag="xt")
                    # padded x: x_in[:, t0 : t0+PAD+T] covers taps t0-3..t0+T-1
                    nc.sync.dma_start(out=xt[:], in_=x_in[cs, t0:t0 + PAD + T])

                    # Split the 4 tap-products across ScalarE (single-tensor
                    # ops with per-partition scale/bias) and VectorE (fused
                    # scalar_tensor_tensor); VectorE is the bottleneck, so it
                    # gets the minimum 3 two-tensor-input combines.
                    p1 = ap.tile([P, T], f32, tag="p1")
                    nc.scalar.activation(  # p1 = x[t-2]*w1 + bias
                        p1[:], xt[:, 1:1 + T],
                        mybir.ActivationFunctionType.Identity,
                        bias=bt[:, 0:1], scale=wt[:, 1:2],
                    )
                    p3 = ap.tile([P, T], f32, tag="p3")
                    nc.scalar.activation(  # p3 = x[t]*w3
                        p3[:], xt[:, 3:3 + T],
                        mybir.ActivationFunctionType.Copy,
                        bias=0.0, scale=wt[:, 3:4],
                    )
                    a01 = ap.tile([P, T], f32, tag="a01")
                    nc.vector.scalar_tensor_tensor(  # a01 = x[t-3]*w0 + p1
                        a01[:], xt[:, 0:T], wt[:, 0:1], p1[:], mult, add,
                    )
                    a23 = ap.tile([P, T], f32, tag="a23")
                    nc.vector.scalar_tensor_tensor(  # a23 = x[t-1]*w2 + p3
                        a23[:], xt[:, 2:2 + T], wt[:, 2:3], p3[:], mult, add,
                    )
                    pre = ap.tile([P, T], f32, tag="pre")
                    nc.vector.tensor_add(pre[:], a01[:], a23[:])

                    yt = yp.tile([P, T], f32, tag="yt")
                    nc.scalar.activation(
                        yt[:], pre[:], mybir.ActivationFunctionType.Silu
                    )
                    nc.sync.dma_start(out=y_out[cs, t0:t0 + T], in_=yt[:])
    nc.finalize()
    return nc


def _get_nc():
    if "nc" not in _cached:
        _cached["nc"] = _build_nc()
    return _cached["nc"]


def _install_ntff_shim():
    """Provide antenv.axon_hooks (absent in this container) so
    run_bass_kernel_spmd(trace=True) can NTFF-profile via the axon .so."""
    import sys
    import types
    if "antenv.axon_hooks" in sys.modules:
        return
    boot_dir = "/root/.axon_site/trn_agent_boot"
    so_path = "/opt/axon/libaxon_pjrt.so"
    if boot_dir not in sys.path:
        sys.path.insert(0, boot_dir)
    try:
        import trn_boot
        hook = trn_boot._ntff_profile_via_ctypes(so_path)
    except Exception:
        hook = None
    mod = types.ModuleType("antenv.axon_hooks")
    state = {"hook": hook}
    mod.set_axon_ntff_profile_hook = lambda h: state.update(hook=h)
    mod.get_axon_ntff_profile_hook = lambda: state["hook"]
    sys.modules["antenv.axon_hooks"] = mod


def kernel(hidden_states, attention_mask, W, b):
    from concourse.bass_utils import run_bass_kernel_spmd

    hidden_states = np.asarray(hidden_states, dtype=np.float32)
    attention_mask = np.asarray(attention_mask, dtype=np.float32)
    W = np.asarray(W, dtype=np.float32)
    b = np.asarray(b, dtype=np.float32)

    mask_is_ones = bool(np.all(attention_mask == 1.0))
    x = hidden_states if mask_is_ones else hidden_states * attention_mask[:, :, None]

    # [B, S, C] -> [B, C, S] contiguous for channel-major per-core slices,
    # left-padded with PAD causal zeros along S
    xT = np.zeros((B, C, PAD + S), dtype=np.float32)
    xT[:, :, PAD:] = x.transpose(0, 2, 1)
    input_state = np.ascontiguousarray(xT[:, :, PAD + S - K:])  # [B, C, K]

    w2d = np.ascontiguousarray(W.reshape(C, K))
    b2d = np.ascontiguousarray(b.reshape(C, 1))

    in_maps = []
    for core in range(N_CORES):
        bi, h = divmod(core, CH_HALVES)
        ch = slice(h * CPC, (h + 1) * CPC)
        in_maps.append({
            "x": np.ascontiguousarray(xT[bi, ch, :]),
            "w": w2d[ch],
            "bias": b2d[ch],
        })

    nc = _get_nc()
    trace = bool(int(os.environ.get("KERNEL_TRACE", "0")))
    if trace:
        _install_ntff_shim()
    res = run_bass_kernel_spmd(
        nc, in_maps, core_ids=list(range(N_CORES)), trace=trace,
        **({"trace_cores": list(range(N_CORES))} if trace else {}),
    )
    _cached["last_result"] = res

    yT = np.empty((B, C, S), dtype=np.float32)
    for core in range(N_CORES):
        bi, h = divmod(core, CH_HALVES)
        yT[bi, h * CPC:(h + 1) * CPC, :] = res.results[core]["y"]
    y = np.ascontiguousarray(yT.transpose(0, 2, 1))
    if not mask_is_ones:
        y *= attention_mask[:, :, None]
    return y, input_state


# revision 7
# speedup vs baseline: 1.2315x; 1.0007x over previous
"""Trainium2 Bass kernel for nn_CausalConvolution (depthwise causal conv1d
+ bias + silu + attention mask, prefill path with conv-state cache output).

Reference computation (fp32):
    x  = hidden_states * mask[:, :, None]          # [B, S, C]
    xT = x.transpose(0, 2, 1)                      # [B, C, S]
    input_state = xT[..., -K:]                     # [B, C, K]
    y  = causal_depthwise_conv(xT, W) + b          # [B, C, S]
    y  = silu(y).transpose(0, 2, 1) * mask[..., None]

Sharding: 8 cores = (batch b in 0..3) x (channel half h in 0..1).  Depthwise
conv is independent per channel so this needs no communication.  Each core
processes x[b, h*1024:(h+1)*1024, :] in the [C, S] (channel-major) layout so
the per-channel filter taps become per-partition scalars:

    acc = x[:, t-3]*w0 + bias          (tensor_scalar,        VectorE)
    acc = x[:, t-2]*w1 + acc           (scalar_tensor_tensor, VectorE)
    acc = x[:, t-1]*w2 + acc           (scalar_tensor_tensor, VectorE)
    acc = x[:, t  ]*w3 + acc           (scalar_tensor_tensor, VectorE)
    y   = silu(acc)                    (activation,           ScalarE)

All arithmetic is fp32; the only deviations from the reference are fp32
summation order and ScalarE's spline silu (~1e-7 rel).

The host marshals [B,S,C] -> per-core [C_core, S] contiguous slices (plain
numpy transpose), feeds the 8 NeuronCores SPMD, and reassembles.  The conv
state cache is an exact slice of the transposed input, produced host-side.
"""

import os
import numpy as np

# Problem constants (hardcoded per harness contract; kernel.py must be
# self-contained).
B, S, C, K = 4, 4096, 2048, 4
N_CORES = 8
CH_HALVES = 2
CPC = C // CH_HALVES          # 1024 channels per core
P = 128                        # SBUF partitions
NBLK = CPC // P                # 8 channel blocks per core
PAD = K - 1                    # causal left padding

_cached = {}


def _build_nc():
    import concourse.bacc as bacc
    import concourse.mybir as mybir
    from concourse.tile import TileContext

    f32 = mybir.dt.float32
    mult = mybir.AluOpType.mult
    add = mybir.AluOpType.add
    nc = bacc.Bacc(None, target_bir_lowering=False)
    # x comes pre-padded from the host: [CPC, PAD + S], first PAD cols zero
    x_in = nc.dram_tensor("x", [CPC, PAD + S], f32, kind="ExternalInput")
    w_in = nc.dram_tensor("w", [CPC, K], f32, kind="ExternalInput")
    b_in = nc.dram_tensor("bias", [CPC, 1], f32, kind="ExternalInput")
    y_out = nc.dram_tensor("y", [CPC, S], f32, kind="ExternalOutput")

    T = 2048                   # seq-chunk per tile
    NCHUNK = S // T
    chunks = [(blk, ci) for blk in range(NBLK) for ci in range(NCHUNK)]
    with TileContext(nc) as tc:
        with (
            tc.tile_pool(name="xp", bufs=3) as xp,
            tc.tile_pool(name="wp", bufs=3) as wp,
            tc.tile_pool(name="ap", bufs=2) as ap,
            tc.tile_pool(name="yp", bufs=3) as yp,
        ):
            # Two-stage software pipeline.  Per linear chunk j we emit:
            #   stage A(j):   DMA x, ScalarE products p1/p3 for chunk j
            #   stage B(j-1): VectorE combines + ScalarE silu + DMA out
            # so ScalarE's FIFO runs ...p1(j) p3(j) silu(j-1)... and never
            # blocks VectorE on the next chunk's products.
            state = {}  # j -> dict of tiles

            def stage_a(j):
                blk, ci = chunks[j]
                c0 = blk * P
                cs = slice(c0, c0 + P)
                t0 = ci * T
                st = {"cs": cs, "t0": t0}
                if ci == 0:
                    wt = wp.tile([P, K], f32, tag="wt")
                    bt = wp.tile([P, 1], f32, tag="bt")
                    nc.sync.dma_start(out=wt[:], in_=w_in[cs, :])
                    nc.sync.dma_start(out=bt[:], in_=b_in[cs, :])
                    st["wt"], st["bt"] = wt, bt
                else:
                    st["wt"], st["bt"] = state[j - 1]["wt"], state[j - 1]["bt"]
                wt, bt = st["wt"], st["bt"]
                xt = xp.tile([P, PAD + T], f32, tag="xt")
                # padded x: x_in[:, t0 : t0+PAD+T] covers taps t0-3..t0+T-1
                nc.sync.dma_start(out=xt[:], in_=x_in[cs, t0:t0 + PAD + T])
                st["xt"] = xt
                # ScalarE tap-products (per-partition scale/bias)
                p1 = ap.tile([P, T], f32, tag="p1")
                nc.scalar.activation(  # p1 = x[t-2]*w1 + bias
                    p1[:], xt[:, 1:1 + T],
                    mybir.ActivationFunctionType.Identity,
                    bias=bt[:, 0:1], scale=wt[:, 1:2],
                )
                p3 = ap.tile([P, T], f32, tag="p3")
                nc.scalar.activation(  # p3 = x[t]*w3
                    p3[:], xt[:, 3:3 + T],
                    mybir.ActivationFunctionType.Copy,
                    bias=0.0, scale=wt[:, 3:4],
                )
                st["p1"], st["p3"] = p1, p3
                state[j] = st

            def stage_b(j):
                st = state.pop(j)
                cs, t0 = st["cs"], st["t0"]
                xt, wt = st["xt"], st["wt"]
                # VectorE combines: 3 two-tensor-input ops (the minimum to
                # merge 4 tap streams on a 2-input engine)
                a01 = ap.tile([P, T], f32, tag="a01")
                nc.vector.scalar_tensor_tensor(  # a01 = x[t-3]*w0 + p1
                    a01[:], xt[:, 0:T], wt[:, 0:1], st["p1"][:], mult, add,
                )
                a23 = ap.tile([P, T], f32, tag="a23")
                nc.vector.scalar_tensor_tensor(  # a23 = x[t-1]*w2 + p3
                    a23[:], xt[:, 2:2 + T], wt[:, 2:3], st["p3"][:], mult, add,
                )
                pre = ap.tile([P, T], f32, tag="pre")
                nc.vector.tensor_add(pre[:], a01[:], a23[:])
                yt = yp.tile([P, T], f32, tag="yt")
                nc.scalar.activation(
                    yt[:], pre[:], mybir.ActivationFunctionType.Silu
                )
                nc.sync.dma_start(out=y_out[cs, t0:t0 + T], in_=yt[:])

            stage_a(0)
            for j in range(1, len(chunks)):
                stage_a(j)
                stage_b(j - 1)
            stage_b(len(chunks) - 1)
    nc.finalize()
    return nc


def _get_nc():
    if "nc" not in _cached:
        _cached["nc"] = _build_nc()
    return _cached["nc"]


def _install_ntff_shim():
    """Provide antenv.axon_hooks (absent in this container) so
    run_bass_kernel_spmd(trace=True) can NTFF-profile via the axon .so."""
    import sys
    import types
    if "antenv.axon_hooks" in sys.modules:
        return
    boot_dir = "/root/.axon_site/trn_agent_boot"
    so_path = "/opt/axon/libaxon_pjrt.so"
    if boot_dir not in sys.path:
        sys.path.insert(0, boot_dir)
    try:
        import trn_boot
        hook = trn_boot._ntff_profile_via_ctypes(so_path)
    except Exception:
        hook = None
    mod = types.ModuleType("antenv.axon_hooks")
    state = {"hook": hook}
    mod.set_axon_ntff_profile_hook = lambda h: state.update(hook=h)
    mod.get_axon_ntff_profile_hook = lambda: state["hook"]
    sys.modules["antenv.axon_hooks"] = mod


def kernel(hidden_states, attention_mask, W, b):
    from concourse.bass_utils import run_bass_kernel_spmd

    hidden_states = np.asarray(hidden_states, dtype=np.float32)
    attention_mask = np.asarray(attention_mask, dtype=np.float32)
    W = np.asarray(W, dtype=np.float32)
    b = np.asarray(b, dtype=np.float32)

    mask_is_ones = bool(np.all(attention_mask == 1.0))
    x = hidden_states if mask_is_ones else hidden_states * attention_mask[:, :, None]

    # [B, S, C] -> [B, C, S] contiguous for channel-major per-core slices,
    # left-padded with PAD causal zeros along S
    xT = np.zeros((B, C, PAD + S), dtype=np.float32)
    xT[:, :, PAD:] = x.transpose(0, 2, 1)
    input_state = np.ascontiguousarray(xT[:, :, PAD + S - K:])  # [B, C, K]

    w2d = np.ascontiguousarray(W.reshape(C, K))
    b2d = np.ascontiguousarray(b.reshape(C, 1))

    in_maps = []
    for core in range(N_CORES):
        bi, h = divmod(core, CH_HALVES)
        ch = slice(h * CPC, (h + 1) * CPC)
        in_maps.append({
            "x": np.ascontiguousarray(xT[bi, ch, :]),
            "w": w2d[ch],
            "bias": b2d[ch],
        })

    nc = _get_nc()
    trace = bool(int(os.environ.get("KERNEL_TRACE", "0")))
    if trace:
        _install_ntff_shim()
    res = run_bass_kernel_spmd(
        nc, in_maps, core_ids=list(range(N_CORES)), trace=trace,
        **({"trace_cores": list(range(N_CORES))} if trace else {}),
    )
    _cached["last_result"] = res

    yT = np.empty((B, C, S), dtype=np.float32)
    for core in range(N_CORES):
        bi, h = divmod(core, CH_HALVES)
        yT[bi, h * CPC:(h + 1) * CPC, :] = res.results[core]["y"]
    y = np.ascontiguousarray(yT.transpose(0, 2, 1))
    if not mask_is_ones:
        y *= attention_mask[:, :, None]
    return y, input_state
